# revision 1
# baseline (speedup 1.0000x reference)
"""MoE (E=8 experts, top-2, D=1024, T=8192) — expert-parallel Trainium2 kernel.

Strategy (per the expert-parallel sharding hint):
  - Host computes the gate (0.1% of FLOPs: scores, top-2, softmax) and uses it
    to shard tokens: each of the 8 NeuronCores owns one expert and receives
    exactly the tokens routed to it (padded to a common capacity C).
  - Each core runs the dense expert FFN + layernorm + combine-weight scaling
    over its routed tokens: 99.9% of the FLOPs.
  - Host gathers the per-expert outputs back into token order (pure gather —
    slot assignment makes a scatter unnecessary) and sums the K=2 contributions.

Device dataflow (activations kept transposed, features on partitions, so the
mm1 -> gelu -> mm2 chain composes with zero transposes):
  xT [D, C] --mm1(w1 streamed bf16)--> hT [2D, C] --gelu+b1--> mm2(w2 resident
  bf16) --> + x residual (f32r stream) --> zT [D, C] --LN--> y*wv --> outT

The FFN matmuls run in bf16 (1 cyc/row on the PE; fp32 is 4x slower); the
residual add and layernorm run on exact fp32 x via the float32r path, keeping
the end-to-end error ~1e-3. LN statistics are computed on the PE itself: an
accumulating ones[128,128].T @ zT matmul sums over the feature (partition)
axis AND broadcasts the result across all 128 partitions in one instruction.
mm1 is software-pipelined one tile ahead of mm2/LN, with mm2 accumulation
groups interleaved between mm1 groups at 1:2 so the in-order PE always has an
independent group queued behind any group waiting on DMA or the DVE.
"""

import sys

sys.path.insert(0, "/opt/trn_rl_repo")

import ml_dtypes
import numpy as np

E, K, D = 8, 2, 1024
H = 2 * D  # ffn hidden
B, S = 2, 4096
T = B * S
CT = 512  # max token tile (matmul moving free dim; fp32 moving cap is 512)
CT_MIN = 256  # remainder tile; >=256 keeps f32r at full PE rate
P = 128

_kernel_cache = {}


def _build_bass(C, use_b2, use_lng, use_lnb):
    """Build the per-core expert-FFN kernel for capacity C (multiple of CT_MIN)."""
    import concourse.tile as tile
    from concourse import bacc, mybir

    f32 = mybir.dt.float32
    f32r = mybir.dt.float32r
    bf16 = mybir.dt.bfloat16
    AF = mybir.ActivationFunctionType
    OP = mybir.AluOpType

    # tile layout: as many 512-token tiles as fit, then one 256 remainder
    assert C % CT_MIN == 0
    tiles = [(i * CT, CT) for i in range(C // CT)]
    if C % CT:
        tiles.append((C - C % CT, C % CT))
    KO1 = D // P  # 8  k-chunks for mm1
    MO1 = H // P  # 16 m-chunks for mm1
    KO2 = H // P  # 16 k-chunks for mm2
    MO2 = D // P  # 8  m-chunks for mm2

    nc = bacc.Bacc("TRN2", target_bir_lowering=False, debug=False)
    xT_d = nc.dram_tensor("xT", [D, C], bf16, kind="ExternalInput")
    xTf_d = nc.dram_tensor("xTf", [D, C], f32r, kind="ExternalInput")
    w1_d = nc.dram_tensor("w1", [D, H], bf16, kind="ExternalInput")
    w2_d = nc.dram_tensor("w2", [H, D], bf16, kind="ExternalInput")
    b1_d = nc.dram_tensor("b1", [H], f32, kind="ExternalInput")
    b2_d = nc.dram_tensor("b2", [D], f32, kind="ExternalInput")
    lng_d = nc.dram_tensor("ln_g", [D], f32, kind="ExternalInput")
    lnb_d = nc.dram_tensor("ln_b", [D], f32, kind="ExternalInput")
    wv_d = nc.dram_tensor("wv", [P, C], f32, kind="ExternalInput")
    out_d = nc.dram_tensor("outT", [D, C], f32, kind="ExternalOutput")

    xT_r = xT_d.rearrange("(ko p) c -> p ko c", p=P)
    xTf_r = xTf_d.rearrange("(ko p) c -> p ko c", p=P)
    w1_r = w1_d.rearrange("(ko p) m -> p ko m", p=P)
    w2_r = w2_d.rearrange("(ko p) m -> p ko m", p=P)
    out_r = out_d.rearrange("(mo p) c -> p mo c", p=P)

    with tile.TileContext(nc) as tc:
        with (
            tc.tile_pool(name="singles", bufs=1) as singles,
            tc.tile_pool(name="xp", bufs=3) as xp,
            tc.tile_pool(name="hp", bufs=2) as hp,
            tc.tile_pool(name="zp", bufs=3) as zp,
            tc.tile_pool(name="zqp", bufs=2) as zqp,
            tc.tile_pool(name="wvp", bufs=2) as wvp,
            tc.tile_pool(name="w1p", bufs=12) as w1p,
            tc.tile_pool(name="xrp", bufs=6) as xrp,
            tc.tile_pool(name="stp", bufs=2) as stp,
            tc.tile_pool(name="ocp", bufs=3) as ocp,
            tc.tile_pool(name="psmm", bufs=6, space="PSUM") as psmm,
            tc.tile_pool(name="psst", bufs=1, space="PSUM") as psst,
        ):
            # ---- resident data ----
            # tile 0's inputs first (SP ring is FIFO): xT(0), then w1 chunk by
            # chunk so tile 0's first matmuls start as soon as chunk 0 lands
            xT_tiles = [xp.tile([P, KO1, CT], bf16, name="xT_sb") for _ in range(3)]
            ct0 = tiles[0][1]
            # split tile 0's xT across both HWDGE rings — the ACT ring is idle
            # at startup, so this halves the latency to the first matmul
            half = KO1 // 2
            nc.sync.dma_start(xT_tiles[0][:, :half, :ct0], xT_r[:, :half, 0:ct0])
            nc.scalar.dma_start(xT_tiles[0][:, half:, :ct0], xT_r[:, half:, 0:ct0])
            # constants go on the ACT HWDGE ring, parallel to SP's
            ones_f32 = singles.tile([P, P], f32)
            nc.vector.memset(ones_f32[:], 1.0)
            ones_sb = singles.tile([P, P], f32r)
            nc.vector.tensor_copy(ones_sb[:], ones_f32[:])
            eps_sb = singles.tile([P, 1], f32)
            nc.vector.memset(eps_sb[:], 1e-6)
            b1_sb = singles.tile([P, MO1], f32)
            nc.scalar.dma_start(b1_sb[:], b1_d.rearrange("(mo p) -> p mo", p=P))
            b2_sb = singles.tile([P, MO2], f32)
            nc.scalar.dma_start(b2_sb[:], b2_d.rearrange("(mo p) -> p mo", p=P))
            lng_sb = singles.tile([P, MO2], f32)
            nc.scalar.dma_start(lng_sb[:], lng_d.rearrange("(mo p) -> p mo", p=P))
            lnb_sb = singles.tile([P, MO2], f32)
            nc.scalar.dma_start(lnb_sb[:], lnb_d.rearrange("(mo p) -> p mo", p=P))
            # w2 resident in bf16 (half the SBUF, fast LDWEIGHTS, no
            # streaming); loaded after tile 0's w1 chunks are queued so it
            # stays off the startup critical path
            w2_sb = singles.tile([P, KO2, D], bf16)

            def emit_w2_load():
                for mo in range(MO2):
                    nc.scalar.dma_start(
                        w2_sb[:, :, mo * P : (mo + 1) * P],
                        w2_r[:, :, mo * P : (mo + 1) * P],
                    )

            def emit_mm1(t, t0, ct, xT_sb, prev_st=None):
                """mm1 + gelu into a fresh hT tile; mm2 groups of the previous
                tile interleave 1:2 so the PE always has an independent
                accumulation group queued behind any stalled one."""
                hT_sb = hp.tile([P, KO2, CT], bf16, name="hT_sb")
                for m in range(MO1):
                    w1c = w1p.tile([P, KO1, P], bf16, name="w1c")
                    nc.sync.dma_start(w1c[:], w1_r[:, :, m * P : (m + 1) * P])
                    ps = psmm.tile([P, CT], f32, name="ps_mm")
                    for k in range(KO1):
                        nc.tensor.matmul(
                            ps[:, :ct],
                            w1c[:, k, :],
                            xT_sb[:, k, :ct],
                            start=(k == 0),
                            stop=(k == KO1 - 1),
                        )
                    nc.scalar.activation(
                        out=hT_sb[:, m, :ct],
                        in_=ps[:, :ct],
                        func=AF.Gelu,
                        bias=b1_sb[:, m : m + 1],
                        scale=1.0,
                    )
                    if prev_st is not None and m % 2 == 1:
                        mm2_group(prev_st, m // 2)
                return hT_sb

            def mm2_begin(t, t0, ct, xT_sb, hT_sb):
                ts = slice(t0, t0 + ct)
                wv_sb = wvp.tile([P, CT], f32, name="wv_sb")[:, :ct]
                nc.scalar.dma_start(wv_sb, wv_d[:, ts])
                return {
                    "t": t, "t0": t0, "ct": ct, "ts": ts, "hT_sb": hT_sb,
                    "wv_sb": wv_sb,
                    "zT_sb": zp.tile([P, MO2, CT], f32r, name="zT_sb"),
                    "ps_S": psst.tile([P, CT], f32, name="ps_S"),
                    "ps_Q": psst.tile([P, CT], f32, name="ps_Q"),
                    "pending": [],
                }

            def mm2_stats(st, mo, zc, zq):
                ct = st["ct"]
                nc.tensor.matmul(
                    st["ps_S"][:, :ct], ones_sb[:], zc,
                    start=(mo == 0), stop=(mo == MO2 - 1), skip_group_check=True,
                )
                nc.tensor.matmul(
                    st["ps_Q"][:, :ct], ones_sb[:], zq,
                    start=(mo == 0), stop=(mo == MO2 - 1), skip_group_check=True,
                )

            def mm2_group(st, mo):
                ct, ts = st["ct"], st["ts"]
                xres = xrp.tile([P, CT], f32r, name="xres")[:, :ct]
                nc.scalar.dma_start(xres, xTf_r[:, mo, ts])
                ps = psmm.tile([P, CT], f32, name="ps_mm")
                for k in range(KO2):
                    nc.tensor.matmul(
                        ps[:, :ct],
                        w2_sb[:, k, mo * P : (mo + 1) * P],
                        st["hT_sb"][:, k, :ct],
                        start=(k == 0),
                        stop=(k == KO2 - 1),
                    )
                if st["pending"]:
                    mm2_stats(st, *st["pending"].pop())
                zc = st["zT_sb"][:, mo, :ct]
                nc.vector.tensor_tensor(zc, ps[:, :ct], xres, OP.add)
                if use_b2:
                    nc.vector.tensor_scalar_add(zc, zc, b2_sb[:, mo : mo + 1])
                zq = zqp.tile([P, CT], f32r, name="zq")[:, :ct]
                nc.vector.tensor_tensor(zq, zc, zc, OP.mult)
                st["pending"].append((mo, zc, zq))

            def ln_tail(st):
                ct, ts, wv_sb = st["ct"], st["ts"], st["wv_sb"]
                zT_sb = st["zT_sb"]
                mm2_stats(st, *st["pending"].pop())
                # LN stats: mean/rstd replicated across partitions
                mean_sb = stp.tile([P, CT], f32, name="mean")[:, :ct]
                nc.vector.tensor_scalar_mul(mean_sb, st["ps_S"][:, :ct], 1.0 / D)
                rw_sb = stp.tile([P, CT], f32, name="rw")[:, :ct]
                nc.vector.tensor_scalar_mul(rw_sb, st["ps_Q"][:, :ct], 1.0 / D)
                msq_sb = zqp.tile([P, CT], f32, name="zq")[:, :ct]
                nc.vector.tensor_tensor(msq_sb, mean_sb, mean_sb, OP.mult)
                nc.vector.tensor_tensor(rw_sb, rw_sb, msq_sb, OP.subtract)
                # rw = wv / sqrt(var + eps): sqrt on ACT, reciprocal on DVE
                nc.scalar.activation(
                    out=rw_sb, in_=rw_sb, func=AF.Sqrt, bias=eps_sb[:], scale=1.0,
                )
                nc.vector.reciprocal(rw_sb, rw_sb)
                nc.vector.tensor_tensor(rw_sb, rw_sb, wv_sb, OP.mult)
                # normalize + scale + store
                for mo in range(MO2):
                    oc = ocp.tile([P, CT], f32, name="oc")[:, :ct]
                    nc.vector.tensor_tensor(oc, zT_sb[:, mo, :ct], mean_sb, OP.subtract)
                    nc.vector.tensor_tensor(oc, oc, rw_sb, OP.mult)
                    if use_lng:
                        nc.vector.tensor_scalar_mul(oc, oc, lng_sb[:, mo : mo + 1])
                    if use_lnb:
                        lb = ocp.tile([P, CT], f32, name="lb")[:, :ct]
                        nc.vector.tensor_scalar_mul(lb, wv_sb, lnb_sb[:, mo : mo + 1])
                        nc.vector.tensor_tensor(oc, oc, lb, OP.add)
                    nc.sync.dma_start(out_r[:, mo, ts], oc)

            # xT(t) is consumed only by mm1(t) (residual uses the xres
            # stream); prefetches are queued after each tile's w1 chunks so
            # they never delay the critical w1 stream
            prev_st = None
            for t, (t0, ct) in enumerate(tiles):
                hT_sb = emit_mm1(t, t0, ct, xT_tiles[t % 3], prev_st)
                if t == 0:
                    emit_w2_load()
                for tn_i in ([1, 2] if t == 0 else [t + 2]):
                    if tn_i < len(tiles):
                        tn, cn = tiles[tn_i]
                        nc.sync.dma_start(
                            xT_tiles[tn_i % 3][:, :, :cn], xT_r[:, :, tn : tn + cn]
                        )
                if prev_st is not None:
                    ln_tail(prev_st)
                prev_st = mm2_begin(t, t0, ct, xT_tiles[t % 3], hT_sb)
            for mo in range(MO2):
                mm2_group(prev_st, mo)
            ln_tail(prev_st)

    nc.finalize()
    return nc


def _route(x, gate_w):
    """Host gate: top-2 per token + softmax combine weights (matches
    jax.lax.top_k tie-breaking: lower index wins)."""
    xt = x.reshape(-1, D)
    scores = xt.astype(np.float32) @ gate_w.astype(np.float32)  # [T, E]
    e0 = np.argmax(scores, axis=1)
    s0 = scores[np.arange(T), e0]
    masked = scores.copy()
    masked[np.arange(T), e0] = -np.inf
    e1 = np.argmax(masked, axis=1)
    s1 = masked[np.arange(T), e1]
    # softmax over the two selected scores
    mx = np.maximum(s0, s1)
    z0 = np.exp((s0 - mx).astype(np.float64))
    z1 = np.exp((s1 - mx).astype(np.float64))
    den = z0 + z1
    w0 = (z0 / den).astype(np.float32)
    w1 = (z1 / den).astype(np.float32)
    return xt, e0, e1, w0, w1


def kernel(x, gate_w, w1, b1, w2, b2, ln_g, ln_b):
    from concourse.bass_utils import run_bass_kernel_spmd

    x = np.asarray(x)
    xt, e0, e1, wk0, wk1 = _route(x, np.asarray(gate_w))

    # slot assignment: expert e's token list = tokens with e0==e, then e1==e
    idx_e, wv_e = [], []
    for e in range(E):
        i0 = np.nonzero(e0 == e)[0]
        i1 = np.nonzero(e1 == e)[0]
        idx_e.append(np.concatenate([i0, i1]))
        wv_e.append(np.concatenate([wk0[i0], wk1[i1]]))
    maxn = max(len(i) for i in idx_e)
    C = max(CT_MIN, -(-maxn // CT_MIN) * CT_MIN)

    use_b2 = bool(np.any(np.asarray(b2) != 0))
    use_lng = bool(np.any(np.asarray(ln_g) != 1))
    use_lnb = bool(np.any(np.asarray(ln_b) != 0))
    key = (C, use_b2, use_lng, use_lnb)
    if key not in _kernel_cache:
        _kernel_cache[key] = _build_bass(C, use_b2, use_lng, use_lnb)
    nc = _kernel_cache[key]

    in_maps = []
    for e in range(E):
        n = len(idx_e[e])
        xTe = np.zeros((D, C), np.float32)
        xTe[:, :n] = xt[idx_e[e]].T
        wve = np.zeros((C,), np.float32)
        wve[:n] = wv_e[e]
        in_maps.append({
            "xT": xTe.astype(ml_dtypes.bfloat16),
            "xTf": xTe,
            "w1": np.ascontiguousarray(np.asarray(w1)[e]).astype(ml_dtypes.bfloat16),
            "w2": np.ascontiguousarray(np.asarray(w2)[e]).astype(ml_dtypes.bfloat16),
            "b1": np.ascontiguousarray(np.asarray(b1)[e]),
            "b2": np.ascontiguousarray(np.asarray(b2)[e]),
            "ln_g": np.ascontiguousarray(np.asarray(ln_g)[e]),
            "ln_b": np.ascontiguousarray(np.asarray(ln_b)[e]),
            "wv": np.broadcast_to(wve, (P, C)).copy(),
        })

    res = run_bass_kernel_spmd(nc, in_maps, core_ids=list(range(E)))
    kernel.last_results = res

    # combine: token t's two contributions live at known (expert, slot) pairs
    slot0 = np.empty(T, np.int64)
    slot1 = np.empty(T, np.int64)
    for e in range(E):
        n0 = int(np.sum(e0 == e))
        slot0[e0 == e] = np.arange(n0)
        slot1[e1 == e] = n0 + np.arange(int(np.sum(e1 == e)))
    Y = np.stack([res.results[e]["outT"] for e in range(E)])  # [E, D, C]
    out = Y[e0, :, slot0] + Y[e1, :, slot1]  # [T, D]
    return out.reshape(x.shape).astype(np.float32)



# revision 4
# speedup vs baseline: 1.8925x; 1.8925x over previous
"""MoE (E=8 experts, top-2, D=1024, T=8192) — expert-parallel Trainium2 kernel.

Strategy (per the expert-parallel sharding hint):
  - Host computes the gate (0.1% of FLOPs) and shards tokens: each of the 8
    NeuronCores owns one expert and receives exactly the tokens routed to it
    (padded to a common capacity C, multiple of 64).
  - Each core runs the dense expert FFN + layernorm + combine-weight scaling
    over its routed tokens (99.9% of the FLOPs).
  - Host gathers the per-expert outputs back into token order and sums the
    K=2 contributions.

v2 (fp8 DoubleRow): both FFN matmuls run in fp8e4m3 with
perf_mode=DoubleRow (two contraction rows per PE cell -> ~1.8x the bf16
matmul rate). Max rel err vs the fp32 reference is ~1.7e-2 (measured), inside
the 2e-2 gate. Both weight matrices are fp8-resident in SBUF (2 MB each), so
there is no per-tile weight streaming at all; per-tile DMA is just the token
activations in fp8 (mm1 operand) + bf16 (exact-enough residual), the combine
weights, and the bf16 output. Biases/LN params are host-prelaid to [P, chunks]
so their DMAs are single-descriptor-per-partition (the on-the-fly rearrange
gathers cost 2-5us of DGE ring time each in v1).

LN statistics: z chunks (bf16) are pre-summed across the 8 feature chunks on
the DVE (and squared on ACT), so the partition reduction is a single bf16
ones-matmul per statistic instead of 16 f32r accumulating matmuls (which
lower to multi-pass fp32 on HW, ~580ns each). rw = wv * Rsqrt(var+eps) uses
the ACT Rsqrt LUT (the DVE reciprocal costs 1.7us+).

Software pipeline per token tile t: mm1(t) m-groups with mm2(t-1) mo-groups
interleaved 1:1 over the first 8 groups, ln_tail(t-1) emitted after m-group 8
so its two stats matmuls land while mm1(t) still has ~7 groups queued (the
DVE pre-sums finish in the shadow).
"""

import sys

sys.path.insert(0, "/opt/trn_rl_repo")

import ml_dtypes
import numpy as np

E, K, D = 8, 2, 1024
H = 2 * D
B, S = 2, 4096
T = B * S
CT = 512  # token tile (matmul moving free dim; fp8 DR moving = 2*CT = 1024 max)
PAD = 64  # capacity padding granularity
P = 128
KO1 = D // P  # 8
MO1 = H // P  # 16
KO2 = H // P  # 16
MO2 = D // P  # 8

_kernel_cache = {}


def _build_bass(C, use_b2, use_lng, use_lnb):
    """Per-core expert-FFN kernel for capacity C (multiple of PAD)."""
    import concourse.tile as tile
    from concourse import bacc, mybir

    f32 = mybir.dt.float32
    bf16 = mybir.dt.bfloat16
    f8 = mybir.dt.float8e4
    AF = mybir.ActivationFunctionType
    OP = mybir.AluOpType
    DR = mybir.MatmulPerfMode.DoubleRow

    assert C % PAD == 0
    tiles = [(i * CT, CT) for i in range(C // CT)]
    if C % CT:
        tiles.append((C - C % CT, C % CT))

    nc = bacc.Bacc("TRN2", target_bir_lowering=False, debug=False)
    x8_d = nc.dram_tensor("x8", [D, C], f8, kind="ExternalInput")
    xb_d = nc.dram_tensor("xb", [D, C], bf16, kind="ExternalInput")
    w1_d = nc.dram_tensor("w1", [D, H], f8, kind="ExternalInput")
    w2_d = nc.dram_tensor("w2", [H, D], f8, kind="ExternalInput")
    b1_d = nc.dram_tensor("b1", [P, MO1], f32, kind="ExternalInput")
    if use_b2:
        b2_d = nc.dram_tensor("b2", [P, MO2], f32, kind="ExternalInput")
    if use_lng:
        lng_d = nc.dram_tensor("ln_g", [P, MO2], f32, kind="ExternalInput")
    if use_lnb:
        lnb_d = nc.dram_tensor("ln_b", [P, MO2], f32, kind="ExternalInput")
    wv_d = nc.dram_tensor("wv", [P, C], f32, kind="ExternalInput")
    out_d = nc.dram_tensor("outT", [D, C], bf16, kind="ExternalOutput")

    x8_r = x8_d.rearrange("(ko p) c -> p ko c", p=P)
    xb_r = xb_d.rearrange("(mo p) c -> p mo c", p=P)
    w1_r = w1_d.rearrange("(ko p) m -> p ko m", p=P)
    w2_r = w2_d.rearrange("(ko p) m -> p ko m", p=P)
    out_r = out_d.rearrange("(mo p) c -> p mo c", p=P)

    with tile.TileContext(nc) as tc:
        with (
            tc.tile_pool(name="singles", bufs=1) as singles,
            tc.tile_pool(name="xp", bufs=3) as xp,
            tc.tile_pool(name="xbp", bufs=3) as xbp,
            tc.tile_pool(name="hp", bufs=2) as hp,
            tc.tile_pool(name="zp", bufs=2) as zp,
            tc.tile_pool(name="zqp", bufs=2) as zqp,
            tc.tile_pool(name="accp", bufs=2) as accp,
            tc.tile_pool(name="wvp", bufs=2) as wvp,
            tc.tile_pool(name="stp", bufs=2) as stp,
            tc.tile_pool(name="ocp", bufs=2) as ocp,
            tc.tile_pool(name="psmm", bufs=6, space="PSUM") as psmm,
            tc.tile_pool(name="psst", bufs=1, space="PSUM") as psst,
        ):
            # ---- startup DMAs: tile 0's inputs first on both rings ----
            ct0 = tiles[0][1]
            x8_tiles = [xp.tile([P, KO1, CT], f8, name="x8_sb") for _ in range(3)]
            xb_tiles = [xbp.tile([P, MO2, CT], bf16, name="xb_sb") for _ in range(3)]
            nc.sync.dma_start(x8_tiles[0][:, :, :ct0], x8_r[:, :, 0:ct0])
            # w1 resident, loaded in 4 m-quarters (ACT ring) so tile 0's first
            # m-groups can start as soon as quarter 0 lands
            w1_sb = singles.tile([P, KO1, H], f8)
            QW = H // 4
            for q in range(4):
                nc.scalar.dma_start(
                    w1_sb[:, :, q * QW : (q + 1) * QW], w1_r[:, :, q * QW : (q + 1) * QW]
                )
            # small constants (contiguous host-prelaid layouts)
            ones_bf = singles.tile([P, P], bf16)
            nc.vector.memset(ones_bf[:], 1.0)
            eps_sb = singles.tile([P, 1], f32)
            nc.vector.memset(eps_sb[:], 1e-6)
            b1_sb = singles.tile([P, MO1], f32)
            nc.sync.dma_start(b1_sb[:], b1_d[:])
            if use_b2:
                b2_sb = singles.tile([P, MO2], f32)
                nc.sync.dma_start(b2_sb[:], b2_d[:])
            if use_lng:
                lng_sb = singles.tile([P, MO2], f32)
                nc.sync.dma_start(lng_sb[:], lng_d[:])
            if use_lnb:
                lnb_sb = singles.tile([P, MO2], f32)
                nc.sync.dma_start(lnb_sb[:], lnb_d[:])
            # residual stream for tile 0 + w2 resident (one 2 MB DMA, 2KB rows)
            nc.sync.dma_start(xb_tiles[0][:, :, :ct0], xb_r[:, :, 0:ct0])
            w2_sb = singles.tile([P, KO2, D], f8)
            nc.scalar.dma_start(w2_sb[:], w2_r[:])

            def emit_mm1(t, ct, prev_st):
                """mm1 + gelu into a fresh fp8 hT tile; the previous tile's
                mm2 groups interleave 1:1 over the first 8 m-groups and its
                ln_tail is emitted after m-group 8."""
                hT_sb = hp.tile([P, KO2, CT], f8, name="hT_sb")
                x8_sb = x8_tiles[t % 3]
                for m in range(MO1):
                    ps = psmm.tile([P, CT], f32, name="ps_mm", tag="mm")
                    for kk in range(KO1 // 2):
                        nc.tensor.matmul(
                            ps[:, :ct],
                            w1_sb[:, 2 * kk : 2 * kk + 2, m * P : (m + 1) * P],
                            x8_sb[:, 2 * kk : 2 * kk + 2, :ct],
                            start=(kk == 0),
                            stop=(kk == KO1 // 2 - 1),
                            perf_mode=DR,
                        )
                    nc.scalar.activation(
                        out=hT_sb[:, m, :ct],
                        in_=ps[:, :ct],
                        func=AF.Gelu,
                        bias=b1_sb[:, m : m + 1],
                        scale=1.0,
                    )
                    if prev_st is not None:
                        if m < MO2:
                            mm2_group(prev_st, m)
                        elif m == MO2:
                            ln_tail(prev_st)
                return hT_sb

            def mm2_begin(t, t0, ct, hT_sb):
                ts = slice(t0, t0 + ct)
                wv_sb = wvp.tile([P, CT], f32, name="wv_sb")[:, :ct]
                nc.scalar.dma_start(wv_sb, wv_d[:, ts])
                return {
                    "t": t, "t0": t0, "ct": ct, "ts": ts, "hT_sb": hT_sb,
                    "wv_sb": wv_sb,
                    "xb_sb": xb_tiles[t % 3],
                    "z_sb": zp.tile([P, MO2, CT], bf16, name="z_sb"),
                    "sacc": accp.tile([P, CT], bf16, name="sacc", tag="sacc"),
                    "qacc": accp.tile([P, CT], bf16, name="qacc", tag="qacc"),
                }

            def mm2_group(st, mo):
                ct = st["ct"]
                ps = psmm.tile([P, CT], f32, name="ps_mm", tag="mm")
                for jj in range(KO2 // 2):
                    nc.tensor.matmul(
                        ps[:, :ct],
                        w2_sb[:, 2 * jj : 2 * jj + 2, mo * P : (mo + 1) * P],
                        st["hT_sb"][:, 2 * jj : 2 * jj + 2, :ct],
                        start=(jj == 0),
                        stop=(jj == KO2 // 2 - 1),
                        perf_mode=DR,
                    )
                zc = st["z_sb"][:, mo, :ct]
                nc.vector.tensor_tensor(zc, ps[:, :ct], st["xb_sb"][:, mo, :ct], OP.add)
                if use_b2:
                    nc.vector.tensor_scalar_add(zc, zc, b2_sb[:, mo : mo + 1])
                zq = zqp.tile([P, CT], bf16, name="zq")[:, :ct]
                nc.scalar.activation(out=zq, in_=zc, func=AF.Square, scale=1.0)
                sacc, qacc = st["sacc"][:, :ct], st["qacc"][:, :ct]
                if mo == 0:
                    nc.vector.tensor_copy(sacc, zc)
                    nc.vector.tensor_copy(qacc, zq)
                else:
                    nc.vector.tensor_tensor(sacc, sacc, zc, OP.add)
                    nc.vector.tensor_tensor(qacc, qacc, zq, OP.add)

            def ln_tail(st):
                ct, ts, wv_sb = st["ct"], st["ts"], st["wv_sb"]
                z_sb = st["z_sb"]
                # partition reduction + broadcast: one bf16 ones-matmul per stat
                ps_S = psst.tile([P, CT], f32, name="ps_S", tag="psS")
                ps_Q = psst.tile([P, CT], f32, name="ps_Q", tag="psQ")
                nc.tensor.matmul(ps_S[:, :ct], ones_bf[:], st["sacc"][:, :ct])
                nc.tensor.matmul(ps_Q[:, :ct], ones_bf[:], st["qacc"][:, :ct])
                mean32 = stp.tile([P, CT], f32, name="mean32", tag="mean32")[:, :ct]
                nc.vector.tensor_scalar_mul(mean32, ps_S[:, :ct], 1.0 / D)
                var32 = stp.tile([P, CT], f32, name="var32", tag="var32")[:, :ct]
                nc.vector.tensor_scalar_mul(var32, ps_Q[:, :ct], 1.0 / D)
                msq = stp.tile([P, CT], f32, name="msq", tag="msq")[:, :ct]
                nc.vector.tensor_tensor(msq, mean32, mean32, OP.mult)
                nc.vector.tensor_tensor(var32, var32, msq, OP.subtract)
                # rw = wv * rsqrt(var + eps). ACT's Rsqrt LUT is blocked for
                # accuracy; rsqrt = Exp(-0.5*Log(var+eps)) via the exp/log
                # LUTs (<=2 ULP each) costs two ACT ops and zero DVE time.
                rs = stp.tile([P, CT], f32, name="rs", tag="rs")[:, :ct]
                nc.scalar.activation(
                    out=rs, in_=var32, func=AF.Ln, bias=eps_sb[:], scale=1.0
                )
                nc.scalar.activation(out=rs, in_=rs, func=AF.Exp, scale=-0.5)
                nc.vector.tensor_tensor(rs, rs, wv_sb, OP.mult)
                rwb = stp.tile([P, CT], bf16, name="rwb", tag="rwb")[:, :ct]
                nc.vector.tensor_copy(rwb, rs)
                meanb = stp.tile([P, CT], bf16, name="meanb", tag="meanb")[:, :ct]
                nc.vector.tensor_copy(meanb, mean32)
                # normalize + scale; stage the whole tile, store with one DMA
                oc = ocp.tile([P, MO2, CT], bf16, name="oc")
                for mo in range(MO2):
                    d = oc[:, mo, :ct]
                    nc.vector.tensor_tensor(d, z_sb[:, mo, :ct], meanb, OP.subtract)
                    nc.vector.tensor_tensor(d, d, rwb, OP.mult)
                    if use_lng:
                        nc.vector.tensor_scalar_mul(d, d, lng_sb[:, mo : mo + 1])
                    if use_lnb:
                        lb = stp.tile([P, CT], f32, name="lb", tag="lb")[:, :ct]
                        nc.vector.tensor_scalar_mul(lb, wv_sb, lnb_sb[:, mo : mo + 1])
                        nc.vector.tensor_tensor(d, d, lb, OP.add)
                nc.sync.dma_start(out_r[:, :, ts], oc[:, :, :ct])

            prev_st = None
            for t, (t0, ct) in enumerate(tiles):
                hT_sb = emit_mm1(t, ct, prev_st)
                # prefetch token tiles two ahead (x8 for mm1, xb for residual)
                for tn_i in ([1, 2] if t == 0 else [t + 2]):
                    if tn_i < len(tiles):
                        tn, cn = tiles[tn_i]
                        nc.sync.dma_start(
                            x8_tiles[tn_i % 3][:, :, :cn], x8_r[:, :, tn : tn + cn]
                        )
                        nc.sync.dma_start(
                            xb_tiles[tn_i % 3][:, :, :cn], xb_r[:, :, tn : tn + cn]
                        )
                prev_st = mm2_begin(t, t0, ct, hT_sb)
            for mo in range(MO2):
                mm2_group(prev_st, mo)
            ln_tail(prev_st)

    nc.finalize()
    return nc


def _route(x, gate_w):
    """Host gate: top-2 per token + softmax combine weights (matches
    jax.lax.top_k tie-breaking: lower index wins)."""
    xt = x.reshape(-1, D)
    scores = xt.astype(np.float32) @ gate_w.astype(np.float32)  # [T, E]
    e0 = np.argmax(scores, axis=1)
    s0 = scores[np.arange(T), e0]
    masked = scores.copy()
    masked[np.arange(T), e0] = -np.inf
    e1 = np.argmax(masked, axis=1)
    s1 = masked[np.arange(T), e1]
    mx = np.maximum(s0, s1)
    z0 = np.exp((s0 - mx).astype(np.float64))
    z1 = np.exp((s1 - mx).astype(np.float64))
    den = z0 + z1
    w0 = (z0 / den).astype(np.float32)
    w1 = (z1 / den).astype(np.float32)
    return xt, e0, e1, w0, w1


def kernel(x, gate_w, w1, b1, w2, b2, ln_g, ln_b):
    from concourse.bass_utils import run_bass_kernel_spmd

    x = np.asarray(x)
    xt, e0, e1, wk0, wk1 = _route(x, np.asarray(gate_w))

    # slot assignment: expert e's token list = tokens with e0==e, then e1==e
    idx_e, wv_e = [], []
    for e in range(E):
        i0 = np.nonzero(e0 == e)[0]
        i1 = np.nonzero(e1 == e)[0]
        idx_e.append(np.concatenate([i0, i1]))
        wv_e.append(np.concatenate([wk0[i0], wk1[i1]]))
    maxn = max(len(i) for i in idx_e)
    C = max(PAD, -(-maxn // PAD) * PAD)

    use_b2 = bool(np.any(np.asarray(b2) != 0))
    use_lng = bool(np.any(np.asarray(ln_g) != 1))
    use_lnb = bool(np.any(np.asarray(ln_b) != 0))
    key = (C, use_b2, use_lng, use_lnb)
    if key not in _kernel_cache:
        _kernel_cache[key] = _build_bass(C, use_b2, use_lng, use_lnb)
    nc = _kernel_cache[key]

    f8 = ml_dtypes.float8_e4m3
    bf = ml_dtypes.bfloat16

    def chunked(a, n):  # [n*P] -> [P, n] host prelayout
        return np.ascontiguousarray(np.asarray(a, np.float32).reshape(n, P).T)

    in_maps = []
    for e in range(E):
        n = len(idx_e[e])
        xTe = np.zeros((D, C), np.float32)
        xTe[:, :n] = xt[idx_e[e]].T
        wve = np.zeros((C,), np.float32)
        wve[:n] = wv_e[e]
        im = {
            "x8": xTe.astype(f8),
            "xb": xTe.astype(bf),
            "w1": np.ascontiguousarray(np.asarray(w1)[e]).astype(f8),
            "w2": np.ascontiguousarray(np.asarray(w2)[e]).astype(f8),
            "b1": chunked(np.asarray(b1)[e], MO1),
            "wv": np.broadcast_to(wve, (P, C)).copy(),
        }
        if use_b2:
            im["b2"] = chunked(np.asarray(b2)[e], MO2)
        if use_lng:
            im["ln_g"] = chunked(np.asarray(ln_g)[e], MO2)
        if use_lnb:
            im["ln_b"] = chunked(np.asarray(ln_b)[e], MO2)
        in_maps.append(im)

    res = run_bass_kernel_spmd(nc, in_maps, core_ids=list(range(E)))
    kernel.last_results = res

    # combine: token t's two contributions live at known (expert, slot) pairs
    slot0 = np.empty(T, np.int64)
    slot1 = np.empty(T, np.int64)
    for e in range(E):
        n0 = int(np.sum(e0 == e))
        slot0[e0 == e] = np.arange(n0)
        slot1[e1 == e] = n0 + np.arange(int(np.sum(e1 == e)))
    Y = np.stack([res.results[e]["outT"].astype(np.float32) for e in range(E)])
    out = Y[e0, :, slot0] + Y[e1, :, slot1]  # [T, D]
    return out.reshape(x.shape).astype(np.float32)


# revision 9
# speedup vs baseline: 1.9434x; 1.0269x over previous
"""MoE (E=8 experts, top-2, D=1024, T=8192) — expert-parallel Trainium2 kernel.

Strategy (per the expert-parallel sharding hint):
  - Host computes the gate (0.1% of FLOPs) and shards tokens: each of the 8
    NeuronCores owns one expert and receives exactly the tokens routed to it
    (padded to a common capacity C, multiple of 64).
  - Each core runs the dense expert FFN + layernorm + combine-weight scaling
    over its routed tokens (99.9% of the FLOPs).
  - Host gathers the per-expert outputs back into token order and sums the
    K=2 contributions.

v2 (fp8 DoubleRow): both FFN matmuls run in fp8e4m3 with
perf_mode=DoubleRow (two contraction rows per PE cell -> ~1.8x the bf16
matmul rate). Max rel err vs the fp32 reference is ~1.7e-2 (measured), inside
the 2e-2 gate. Both weight matrices are fp8-resident in SBUF (2 MB each), so
there is no per-tile weight streaming at all; per-tile DMA is just the token
activations in fp8 (mm1 operand) + bf16 (exact-enough residual), the combine
weights, and the bf16 output. Biases/LN params are host-prelaid to [P, chunks]
so their DMAs are single-descriptor-per-partition (the on-the-fly rearrange
gathers cost 2-5us of DGE ring time each in v1).

LN statistics: z chunks (bf16) are pre-summed across the 8 feature chunks on
the DVE (and squared on ACT), so the partition reduction is a single bf16
ones-matmul per statistic instead of 16 f32r accumulating matmuls (which
lower to multi-pass fp32 on HW, ~580ns each). rw = wv * Rsqrt(var+eps) uses
the ACT Rsqrt LUT (the DVE reciprocal costs 1.7us+).

Software pipeline per token tile t: mm1(t) m-groups with mm2(t-1) mo-groups
interleaved 1:1 over the first 8 groups, ln_tail(t-1) emitted after m-group 8
so its two stats matmuls land while mm1(t) still has ~7 groups queued (the
DVE pre-sums finish in the shadow).
"""

import sys

sys.path.insert(0, "/opt/trn_rl_repo")

import ml_dtypes
import numpy as np

E, K, D = 8, 2, 1024
H = 2 * D
B, S = 2, 4096
T = B * S
CT = 512  # token tile (matmul moving free dim; fp8 DR moving = 2*CT = 1024 max)
PAD = 64  # capacity padding granularity
P = 128
KO1 = D // P  # 8
MO1 = H // P  # 16
KO2 = H // P  # 16
MO2 = D // P  # 8

_kernel_cache = {}


def _build_bass(C, use_b2, use_lng, use_lnb):
    """Per-core expert-FFN kernel for capacity C (multiple of PAD)."""
    import concourse.tile as tile
    from concourse import bacc, mybir

    f32 = mybir.dt.float32
    bf16 = mybir.dt.bfloat16
    f8 = mybir.dt.float8e4
    AF = mybir.ActivationFunctionType
    OP = mybir.AluOpType
    DR = mybir.MatmulPerfMode.DoubleRow

    assert C % PAD == 0
    # first tile 448 (fast startup DMA), full 512s, remainder last (short
    # drain: the final ln_tail + store scale with the last tile's size)
    sizes = []
    rest = C
    if rest > CT:
        sizes.append(448)
        rest -= 448
    sizes += [CT] * (rest // CT)
    if rest % CT:
        sizes.append(rest % CT)
    tiles = []
    t0 = 0
    for ct in sizes:
        tiles.append((t0, ct))
        t0 += ct

    nc = bacc.Bacc("TRN2", target_bir_lowering=False, debug=False)
    x8_d = nc.dram_tensor("x8", [D, C], f8, kind="ExternalInput")
    xb_d = nc.dram_tensor("xb", [D, C], bf16, kind="ExternalInput")
    w1_d = nc.dram_tensor("w1", [D, H], f8, kind="ExternalInput")
    w2_d = nc.dram_tensor("w2", [H, D], f8, kind="ExternalInput")
    b1_d = nc.dram_tensor("b1", [P, MO1], f32, kind="ExternalInput")
    if use_b2:
        b2_d = nc.dram_tensor("b2", [P, MO2], f32, kind="ExternalInput")
    if use_lng:
        lng_d = nc.dram_tensor("ln_g", [P, MO2], f32, kind="ExternalInput")
    if use_lnb:
        lnb_d = nc.dram_tensor("ln_b", [P, MO2], f32, kind="ExternalInput")
    wv_d = nc.dram_tensor("wv", [P, C], f32, kind="ExternalInput")
    out_d = nc.dram_tensor("outT", [D, C], bf16, kind="ExternalOutput")

    x8_r = x8_d.rearrange("(ko p) c -> p ko c", p=P)
    xb_r = xb_d.rearrange("(mo p) c -> p mo c", p=P)
    w1_r = w1_d.rearrange("(ko p) m -> p ko m", p=P)
    w2_r = w2_d.rearrange("(ko p) m -> p ko m", p=P)
    out_r = out_d.rearrange("(mo p) c -> p mo c", p=P)

    with tile.TileContext(nc) as tc:
        with (
            tc.tile_pool(name="singles", bufs=1) as singles,
            tc.tile_pool(name="xp", bufs=3) as xp,
            tc.tile_pool(name="xbp", bufs=3) as xbp,
            tc.tile_pool(name="hp", bufs=2) as hp,
            tc.tile_pool(name="zp", bufs=2) as zp,
            tc.tile_pool(name="zqp", bufs=2) as zqp,
            tc.tile_pool(name="accp", bufs=2) as accp,
            tc.tile_pool(name="wvp", bufs=2) as wvp,
            tc.tile_pool(name="stp", bufs=2) as stp,
            tc.tile_pool(name="ocp", bufs=2) as ocp,
            tc.tile_pool(name="psmm", bufs=6, space="PSUM") as psmm,
            tc.tile_pool(name="psst", bufs=1, space="PSUM") as psst,
        ):
            # ---- startup DMAs: tile 0's inputs first on both rings ----
            ct0 = tiles[0][1]
            x8_tiles = [xp.tile([P, KO1, CT], f8, name="x8_sb") for _ in range(3)]
            xb_tiles = [xbp.tile([P, MO2, CT], bf16, name="xb_sb") for _ in range(3)]
            nc.sync.dma_start(x8_tiles[0][:, :, :ct0], x8_r[:, :, 0:ct0])
            # w1 resident: first 2 m-groups as a small slice so tile 0's
            # first matmuls start early, then the rest in two large loads
            w1_sb = singles.tile([P, KO1, H], f8)
            for lo, hi in ((0, 256), (256, 1024), (1024, 2048)):
                nc.scalar.dma_start(w1_sb[:, :, lo:hi], w1_r[:, :, lo:hi])
            # small constants (contiguous host-prelaid layouts)
            ones_bf = singles.tile([P, P], bf16)
            nc.vector.memset(ones_bf[:], 1.0)
            eps_sb = singles.tile([P, 1], f32)
            nc.vector.memset(eps_sb[:], 1e-6)
            b1_sb = singles.tile([P, MO1], f32)
            nc.sync.dma_start(b1_sb[:], b1_d[:])
            # warm the PE's HAM clock gate during the startup DMA wait: ~3.4us
            # of dummy matmuls on constants so the first real matmuls run at
            # 2.4 GHz instead of 1.2 (the activity window is free-running)
            warm_bf = singles.tile([P, CT], bf16)
            nc.vector.memset(warm_bf[:], 0.0)
            for _ in range(8):
                ps_w = psmm.tile([P, CT], f32, name="ps_w", tag="mm")
                nc.tensor.matmul(ps_w[:], ones_bf[:], warm_bf[:])
            if use_b2:
                b2_sb = singles.tile([P, MO2], f32)
                nc.sync.dma_start(b2_sb[:], b2_d[:])
            if use_lng:
                lng_sb = singles.tile([P, MO2], f32)
                nc.sync.dma_start(lng_sb[:], lng_d[:])
            if use_lnb:
                lnb_sb = singles.tile([P, MO2], f32)
                nc.sync.dma_start(lnb_sb[:], lnb_d[:])
            # residual stream for tile 0 + w2 resident (one 2 MB DMA, 2KB rows)
            nc.sync.dma_start(xb_tiles[0][:, :, :ct0], xb_r[:, :, 0:ct0])
            w2_sb = singles.tile([P, KO2, D], f8)
            nc.scalar.dma_start(w2_sb[:], w2_r[:])

            def emit_mm1(t, ct, prev_st):
                """mm1 + gelu into a fresh fp8 hT tile; the previous tile's
                mm2 groups interleave 1:2 at even m-groups (the last one ends
                an mm1-group before the stats matmuls, hiding DVE latency)."""
                hT_sb = hp.tile([P, KO2, CT], f8, name="hT_sb")
                x8_sb = x8_tiles[t % 3]
                for m in range(MO1):
                    ps = psmm.tile([P, CT], f32, name="ps_mm", tag="mm")
                    for kk in range(KO1 // 2):
                        nc.tensor.matmul(
                            ps[:, :ct],
                            w1_sb[:, 2 * kk : 2 * kk + 2, m * P : (m + 1) * P],
                            x8_sb[:, 2 * kk : 2 * kk + 2, :ct],
                            start=(kk == 0),
                            stop=(kk == KO1 // 2 - 1),
                            perf_mode=DR,
                        )
                    nc.scalar.activation(
                        out=hT_sb[:, m, :ct],
                        in_=ps[:, :ct],
                        func=AF.Gelu,
                        bias=b1_sb[:, m : m + 1],
                        scale=1.0,
                    )
                    if prev_st is not None and m % 2 == 0:
                        mm2_group(prev_st, m // 2)
                return hT_sb

            def mm2_begin(t, t0, ct, hT_sb):
                ts = slice(t0, t0 + ct)
                wv_sb = wvp.tile([P, CT], f32, name="wv_sb")[:, :ct]
                nc.scalar.dma_start(wv_sb, wv_d[:, ts])
                return {
                    "t": t, "t0": t0, "ct": ct, "ts": ts, "hT_sb": hT_sb,
                    "wv_sb": wv_sb,
                    "xb_sb": xb_tiles[t % 3],
                    "z_sb": zp.tile([P, MO2, CT], bf16, name="z_sb"),
                    "sacc": accp.tile([P, CT], bf16, name="sacc", tag="sacc"),
                    "qacc": accp.tile([P, CT], bf16, name="qacc", tag="qacc"),
                }

            def mm2_group(st, mo):
                ct = st["ct"]
                ps = psmm.tile([P, CT], f32, name="ps_mm", tag="mm")
                for jj in range(KO2 // 2):
                    nc.tensor.matmul(
                        ps[:, :ct],
                        w2_sb[:, 2 * jj : 2 * jj + 2, mo * P : (mo + 1) * P],
                        st["hT_sb"][:, 2 * jj : 2 * jj + 2, :ct],
                        start=(jj == 0),
                        stop=(jj == KO2 // 2 - 1),
                        perf_mode=DR,
                    )
                zc = st["z_sb"][:, mo, :ct]
                nc.vector.tensor_tensor(zc, ps[:, :ct], st["xb_sb"][:, mo, :ct], OP.add)
                if use_b2:
                    nc.vector.tensor_scalar_add(zc, zc, b2_sb[:, mo : mo + 1])
                zq = zqp.tile([P, CT], bf16, name="zq")[:, :ct]
                nc.scalar.activation(out=zq, in_=zc, func=AF.Square, scale=1.0)
                sacc, qacc = st["sacc"][:, :ct], st["qacc"][:, :ct]
                if mo == 0:
                    nc.vector.tensor_copy(sacc, zc)
                    nc.vector.tensor_copy(qacc, zq)
                else:
                    nc.vector.tensor_tensor(sacc, sacc, zc, OP.add)
                    nc.vector.tensor_tensor(qacc, qacc, zq, OP.add)

            def ln_tail(st):
                ct, ts, wv_sb = st["ct"], st["ts"], st["wv_sb"]
                z_sb = st["z_sb"]
                # partition reduction + broadcast: one bf16 ones-matmul per stat
                ps_S = psst.tile([P, CT], f32, name="ps_S", tag="psS")
                ps_Q = psst.tile([P, CT], f32, name="ps_Q", tag="psQ")
                nc.tensor.matmul(ps_S[:, :ct], ones_bf[:], st["sacc"][:, :ct])
                nc.tensor.matmul(ps_Q[:, :ct], ones_bf[:], st["qacc"][:, :ct])
                mean32 = stp.tile([P, CT], f32, name="mean32", tag="mean32")[:, :ct]
                nc.vector.tensor_scalar_mul(mean32, ps_S[:, :ct], 1.0 / D)
                var32 = stp.tile([P, CT], f32, name="var32", tag="var32")[:, :ct]
                nc.vector.tensor_scalar_mul(var32, ps_Q[:, :ct], 1.0 / D)
                msq = stp.tile([P, CT], f32, name="msq", tag="msq")[:, :ct]
                nc.vector.tensor_tensor(msq, mean32, mean32, OP.mult)
                nc.vector.tensor_tensor(var32, var32, msq, OP.subtract)
                # rw = wv * rsqrt(var + eps). Abs_reciprocal_sqrt is a single
                # ACT function in a single table set (var+eps > 0, so the abs
                # is a no-op); Ln+Exp spans two sets and pays two extra
                # 1.28us table loads per tile.
                rs = stp.tile([P, CT], f32, name="rs", tag="rs")[:, :ct]
                nc.scalar.activation(
                    out=rs, in_=var32, func=AF.Abs_reciprocal_sqrt,
                    bias=eps_sb[:], scale=1.0,
                )
                nc.vector.tensor_tensor(rs, rs, wv_sb, OP.mult)
                rwb = stp.tile([P, CT], bf16, name="rwb", tag="rwb")[:, :ct]
                nc.vector.tensor_copy(rwb, rs)
                meanb = stp.tile([P, CT], bf16, name="meanb", tag="meanb")[:, :ct]
                nc.vector.tensor_copy(meanb, mean32)
                # normalize + scale; stage the whole tile, store with one DMA
                oc = ocp.tile([P, MO2, CT], bf16, name="oc")
                for mo in range(MO2):
                    d = oc[:, mo, :ct]
                    nc.vector.tensor_tensor(d, z_sb[:, mo, :ct], meanb, OP.subtract)
                    nc.vector.tensor_tensor(d, d, rwb, OP.mult)
                    if use_lng:
                        nc.vector.tensor_scalar_mul(d, d, lng_sb[:, mo : mo + 1])
                    if use_lnb:
                        lb = stp.tile([P, CT], f32, name="lb", tag="lb")[:, :ct]
                        nc.vector.tensor_scalar_mul(lb, wv_sb, lnb_sb[:, mo : mo + 1])
                        nc.vector.tensor_tensor(d, d, lb, OP.add)
                nc.sync.dma_start(out_r[:, :, ts], oc[:, :, :ct])

            prev_st = None
            for t, (t0, ct) in enumerate(tiles):
                hT_sb = emit_mm1(t, ct, prev_st)
                if prev_st is not None:
                    # emitted after the full mm1 so the ACT table switch for
                    # the rsqrt never sits between gelus the next mm2 needs
                    ln_tail(prev_st)
                # prefetch token tiles two ahead (x8 for mm1, xb for residual)
                for tn_i in ([1, 2] if t == 0 else [t + 2]):
                    if tn_i < len(tiles):
                        tn, cn = tiles[tn_i]
                        nc.sync.dma_start(
                            x8_tiles[tn_i % 3][:, :, :cn], x8_r[:, :, tn : tn + cn]
                        )
                        nc.sync.dma_start(
                            xb_tiles[tn_i % 3][:, :, :cn], xb_r[:, :, tn : tn + cn]
                        )
                prev_st = mm2_begin(t, t0, ct, hT_sb)
            for mo in range(MO2):
                mm2_group(prev_st, mo)
            ln_tail(prev_st)

    nc.finalize()
    return nc


def _route(x, gate_w):
    """Host gate: top-2 per token + softmax combine weights (matches
    jax.lax.top_k tie-breaking: lower index wins)."""
    xt = x.reshape(-1, D)
    scores = xt.astype(np.float32) @ gate_w.astype(np.float32)  # [T, E]
    e0 = np.argmax(scores, axis=1)
    s0 = scores[np.arange(T), e0]
    masked = scores.copy()
    masked[np.arange(T), e0] = -np.inf
    e1 = np.argmax(masked, axis=1)
    s1 = masked[np.arange(T), e1]
    mx = np.maximum(s0, s1)
    z0 = np.exp((s0 - mx).astype(np.float64))
    z1 = np.exp((s1 - mx).astype(np.float64))
    den = z0 + z1
    w0 = (z0 / den).astype(np.float32)
    w1 = (z1 / den).astype(np.float32)
    return xt, e0, e1, w0, w1


def kernel(x, gate_w, w1, b1, w2, b2, ln_g, ln_b):
    from concourse.bass_utils import run_bass_kernel_spmd

    x = np.asarray(x)
    xt, e0, e1, wk0, wk1 = _route(x, np.asarray(gate_w))

    # slot assignment: expert e's token list = tokens with e0==e, then e1==e
    idx_e, wv_e = [], []
    for e in range(E):
        i0 = np.nonzero(e0 == e)[0]
        i1 = np.nonzero(e1 == e)[0]
        idx_e.append(np.concatenate([i0, i1]))
        wv_e.append(np.concatenate([wk0[i0], wk1[i1]]))
    maxn = max(len(i) for i in idx_e)
    C = max(PAD, -(-maxn // PAD) * PAD)

    use_b2 = bool(np.any(np.asarray(b2) != 0))
    use_lng = bool(np.any(np.asarray(ln_g) != 1))
    use_lnb = bool(np.any(np.asarray(ln_b) != 0))
    key = (C, use_b2, use_lng, use_lnb)
    if key not in _kernel_cache:
        _kernel_cache[key] = _build_bass(C, use_b2, use_lng, use_lnb)
    nc = _kernel_cache[key]

    f8 = ml_dtypes.float8_e4m3
    bf = ml_dtypes.bfloat16

    def chunked(a, n):  # [n*P] -> [P, n] host prelayout
        return np.ascontiguousarray(np.asarray(a, np.float32).reshape(n, P).T)

    in_maps = []
    for e in range(E):
        n = len(idx_e[e])
        xTe = np.zeros((D, C), np.float32)
        xTe[:, :n] = xt[idx_e[e]].T
        wve = np.zeros((C,), np.float32)
        wve[:n] = wv_e[e]
        im = {
            "x8": xTe.astype(f8),
            "xb": xTe.astype(bf),
            "w1": np.ascontiguousarray(np.asarray(w1)[e]).astype(f8),
            "w2": np.ascontiguousarray(np.asarray(w2)[e]).astype(f8),
            "b1": chunked(np.asarray(b1)[e], MO1),
            "wv": np.broadcast_to(wve, (P, C)).copy(),
        }
        if use_b2:
            im["b2"] = chunked(np.asarray(b2)[e], MO2)
        if use_lng:
            im["ln_g"] = chunked(np.asarray(ln_g)[e], MO2)
        if use_lnb:
            im["ln_b"] = chunked(np.asarray(ln_b)[e], MO2)
        in_maps.append(im)

    res = run_bass_kernel_spmd(nc, in_maps, core_ids=list(range(E)))
    kernel.last_results = res

    # combine: token t's two contributions live at known (expert, slot) pairs
    slot0 = np.empty(T, np.int64)
    slot1 = np.empty(T, np.int64)
    for e in range(E):
        n0 = int(np.sum(e0 == e))
        slot0[e0 == e] = np.arange(n0)
        slot1[e1 == e] = n0 + np.arange(int(np.sum(e1 == e)))
    Y = np.stack([res.results[e]["outT"].astype(np.float32) for e in range(E)])
    out = Y[e0, :, slot0] + Y[e1, :, slot1]  # [T, D]
    return out.reshape(x.shape).astype(np.float32)


# revision 11
# speedup vs baseline: 1.9584x; 1.0077x over previous
"""MoE (E=8 experts, top-2, D=1024, T=8192) — expert-parallel Trainium2 kernel.

Strategy (per the expert-parallel sharding hint):
  - Host computes the gate (0.1% of FLOPs) and shards tokens: each of the 8
    NeuronCores owns one expert and receives exactly the tokens routed to it
    (padded to a common capacity C, multiple of 64).
  - Each core runs the dense expert FFN + layernorm + combine-weight scaling
    over its routed tokens (99.9% of the FLOPs).
  - Host gathers the per-expert outputs back into token order and sums the
    K=2 contributions.

v2 (fp8 DoubleRow): both FFN matmuls run in fp8e4m3 with
perf_mode=DoubleRow (two contraction rows per PE cell -> ~1.8x the bf16
matmul rate). Max rel err vs the fp32 reference is ~1.7e-2 (measured), inside
the 2e-2 gate. Both weight matrices are fp8-resident in SBUF (2 MB each), so
there is no per-tile weight streaming at all; per-tile DMA is just the token
activations in fp8 (mm1 operand) + bf16 (exact-enough residual), the combine
weights, and the bf16 output. Biases/LN params are host-prelaid to [P, chunks]
so their DMAs are single-descriptor-per-partition (the on-the-fly rearrange
gathers cost 2-5us of DGE ring time each in v1).

LN statistics: z chunks (bf16) are pre-summed across the 8 feature chunks on
the DVE (and squared on ACT), so the partition reduction is a single bf16
ones-matmul per statistic instead of 16 f32r accumulating matmuls (which
lower to multi-pass fp32 on HW, ~580ns each). rw = wv * Rsqrt(var+eps) uses
the ACT Rsqrt LUT (the DVE reciprocal costs 1.7us+).

Software pipeline per token tile t: mm1(t) m-groups with mm2(t-1) mo-groups
interleaved 1:1 over the first 8 groups, ln_tail(t-1) emitted after m-group 8
so its two stats matmuls land while mm1(t) still has ~7 groups queued (the
DVE pre-sums finish in the shadow).
"""

import sys

sys.path.insert(0, "/opt/trn_rl_repo")

import ml_dtypes
import numpy as np

E, K, D = 8, 2, 1024
H = 2 * D
B, S = 2, 4096
T = B * S
CT = 512  # token tile (matmul moving free dim; fp8 DR moving = 2*CT = 1024 max)
PAD = 64  # capacity padding granularity
P = 128
KO1 = D // P  # 8
MO1 = H // P  # 16
KO2 = H // P  # 16
MO2 = D // P  # 8

_kernel_cache = {}


def _build_bass(C, use_b2, use_lng, use_lnb):
    """Per-core expert-FFN kernel for capacity C (multiple of PAD)."""
    import concourse.tile as tile
    from concourse import bacc, bass, mybir

    f32 = mybir.dt.float32
    bf16 = mybir.dt.bfloat16
    f8 = mybir.dt.float8e4
    AF = mybir.ActivationFunctionType
    OP = mybir.AluOpType
    DR = mybir.MatmulPerfMode.DoubleRow

    assert C % PAD == 0
    # first tile 448 (fast startup DMA), full 512s, remainder last (short
    # drain: the final ln_tail + store scale with the last tile's size)
    sizes = []
    rest = C
    if rest > CT:
        sizes.append(448)
        rest -= 448
    sizes += [CT] * (rest // CT)
    if rest % CT:
        sizes.append(rest % CT)
    tiles = []
    t0 = 0
    for ct in sizes:
        tiles.append((t0, ct))
        t0 += ct

    nc = bacc.Bacc("TRN2", target_bir_lowering=False, debug=False)
    x8_d = nc.dram_tensor("x8", [D, C], f8, kind="ExternalInput")
    xb_d = nc.dram_tensor("xb", [D, C], bf16, kind="ExternalInput")
    w1_d = nc.dram_tensor("w1", [D, H], f8, kind="ExternalInput")
    w2_d = nc.dram_tensor("w2", [H, D], f8, kind="ExternalInput")
    b1_d = nc.dram_tensor("b1", [P, MO1], f32, kind="ExternalInput")
    if use_b2:
        b2_d = nc.dram_tensor("b2", [P, MO2], f32, kind="ExternalInput")
    if use_lng:
        lng_d = nc.dram_tensor("ln_g", [P, MO2], f32, kind="ExternalInput")
    if use_lnb:
        lnb_d = nc.dram_tensor("ln_b", [P, MO2], f32, kind="ExternalInput")
    wv_d = nc.dram_tensor("wv", [P, C], f32, kind="ExternalInput")
    out_d = nc.dram_tensor("outT", [D, C], bf16, kind="ExternalOutput")

    x8_r = x8_d.rearrange("(ko p) c -> p ko c", p=P)
    xb_r = xb_d.rearrange("(mo p) c -> p mo c", p=P)
    w1_r = w1_d.rearrange("(ko p) m -> p ko m", p=P)
    w2_r = w2_d.rearrange("(ko p) m -> p ko m", p=P)
    out_r = out_d.rearrange("(mo p) c -> p mo c", p=P)

    with tile.TileContext(nc) as tc:
        with (
            tc.tile_pool(name="singles", bufs=1) as singles,
            tc.tile_pool(name="xp", bufs=3) as xp,
            tc.tile_pool(name="xbp", bufs=3) as xbp,
            tc.tile_pool(name="hp", bufs=2) as hp,
            tc.tile_pool(name="zp", bufs=2) as zp,
            tc.tile_pool(name="zqp", bufs=2) as zqp,
            tc.tile_pool(name="accp", bufs=2) as accp,
            tc.tile_pool(name="wvp", bufs=2) as wvp,
            tc.tile_pool(name="stp", bufs=2) as stp,
            tc.tile_pool(name="ocp", bufs=2) as ocp,
            tc.tile_pool(name="psmm", bufs=6, space="PSUM") as psmm,
            tc.tile_pool(name="psst", bufs=1, space="PSUM") as psst,
        ):
            # ---- startup DMAs: tile 0's inputs first on both rings ----
            ct0 = tiles[0][1]
            x8_tiles = [xp.tile([P, KO1, CT], f8, name="x8_sb") for _ in range(3)]
            xb_tiles = [xbp.tile([P, MO2, CT], bf16, name="xb_sb") for _ in range(3)]
            nc.sync.dma_start(x8_tiles[0][:, :, :ct0], x8_r[:, :, 0:ct0])
            # w1 resident: first 2 m-groups as a small slice so tile 0's
            # first matmuls start early, then the rest in two large loads
            w1_sb = singles.tile([P, KO1, H], f8)
            for lo, hi in ((0, 256), (256, 1024), (1024, 2048)):
                nc.scalar.dma_start(w1_sb[:, :, lo:hi], w1_r[:, :, lo:hi])
            # small constants (contiguous host-prelaid layouts)
            ones_bf = singles.tile([P, P], bf16)
            nc.vector.memset(ones_bf[:], 1.0)
            eps_sb = singles.tile([P, 1], f32)
            nc.vector.memset(eps_sb[:], 1e-6)
            b1_sb = singles.tile([P, MO1], f32)
            nc.sync.dma_start(b1_sb[:], b1_d[:])
            # warm the PE's HAM clock gate during the startup DMA wait: ~3.4us
            # of dummy matmuls on constants so the first real matmuls run at
            # 2.4 GHz instead of 1.2 (the activity window is free-running)
            warm_bf = singles.tile([P, CT], bf16)
            nc.vector.memset(warm_bf[:], 0.0)
            for _ in range(8):
                ps_w = psmm.tile([P, CT], f32, name="ps_w", tag="mm")
                nc.tensor.matmul(ps_w[:], ones_bf[:], warm_bf[:])
            if use_b2:
                b2_sb = singles.tile([P, MO2], f32)
                nc.sync.dma_start(b2_sb[:], b2_d[:])
            if use_lng:
                lng_sb = singles.tile([P, MO2], f32)
                nc.sync.dma_start(lng_sb[:], lng_d[:])
            if use_lnb:
                lnb_sb = singles.tile([P, MO2], f32)
                nc.sync.dma_start(lnb_sb[:], lnb_d[:])
            # residual stream for tile 0 + w2 resident (one 2 MB DMA, 2KB rows)
            nc.sync.dma_start(xb_tiles[0][:, :, :ct0], xb_r[:, :, 0:ct0])
            w2_sb = singles.tile([P, KO2, D], f8)
            nc.scalar.dma_start(w2_sb[:], w2_r[:])

            def emit_mm1(t, ct, prev_st):
                """mm1 + gelu into a fresh fp8 hT tile; the previous tile's
                mm2 groups interleave 1:2 at even m-groups (the last one ends
                an mm1-group before the stats matmuls, hiding DVE latency)."""
                hT_sb = hp.tile([P, KO2, CT], f8, name="hT_sb")
                x8_sb = x8_tiles[t % 3]
                for m in range(MO1):
                    ps = psmm.tile([P, CT], f32, name="ps_mm", tag="mm")
                    for kk in range(KO1 // 2):
                        nc.tensor.matmul(
                            ps[:, :ct],
                            w1_sb[:, 2 * kk : 2 * kk + 2, m * P : (m + 1) * P],
                            x8_sb[:, 2 * kk : 2 * kk + 2, :ct],
                            start=(kk == 0),
                            stop=(kk == KO1 // 2 - 1),
                            perf_mode=DR,
                        )
                    nc.scalar.activation(
                        out=hT_sb[:, m, :ct],
                        in_=ps[:, :ct],
                        func=AF.Gelu,
                        bias=b1_sb[:, m : m + 1],
                        scale=1.0,
                    )
                    if prev_st is not None and m % 2 == 0:
                        mm2_group(prev_st, m // 2)
                return hT_sb

            def mm2_begin(t, t0, ct, hT_sb):
                ts = slice(t0, t0 + ct)
                wv_sb = wvp.tile([P, CT], f32, name="wv_sb")[:, :ct]
                nc.scalar.dma_start(wv_sb, wv_d[:, ts])
                return {
                    "t": t, "t0": t0, "ct": ct, "ts": ts, "hT_sb": hT_sb,
                    "wv_sb": wv_sb,
                    "xb_sb": xb_tiles[t % 3],
                    "z_sb": zp.tile([P, MO2, CT], bf16, name="z_sb"),
                    "sacc": accp.tile([P, CT], bf16, name="sacc", tag="sacc"),
                    "qacc": accp.tile([P, CT], bf16, name="qacc", tag="qacc"),
                }

            def mm2_group(st, mo):
                ct = st["ct"]
                ps = psmm.tile([P, CT], f32, name="ps_mm", tag="mm")
                for jj in range(KO2 // 2):
                    nc.tensor.matmul(
                        ps[:, :ct],
                        w2_sb[:, 2 * jj : 2 * jj + 2, mo * P : (mo + 1) * P],
                        st["hT_sb"][:, 2 * jj : 2 * jj + 2, :ct],
                        start=(jj == 0),
                        stop=(jj == KO2 // 2 - 1),
                        perf_mode=DR,
                    )
                zc = st["z_sb"][:, mo, :ct]
                nc.vector.tensor_tensor(zc, ps[:, :ct], st["xb_sb"][:, mo, :ct], OP.add)
                if use_b2:
                    nc.vector.tensor_scalar_add(zc, zc, b2_sb[:, mo : mo + 1])
                zq = zqp.tile([P, CT], bf16, name="zq")[:, :ct]
                nc.scalar.activation(out=zq, in_=zc, func=AF.Square, scale=1.0)
                sacc, qacc = st["sacc"][:, :ct], st["qacc"][:, :ct]
                if mo == 0:
                    nc.vector.tensor_copy(sacc, zc)
                    nc.vector.tensor_copy(qacc, zq)
                else:
                    nc.vector.tensor_tensor(sacc, sacc, zc, OP.add)
                    nc.vector.tensor_tensor(qacc, qacc, zq, OP.add)

            def ln_tail(st):
                ct, ts, wv_sb = st["ct"], st["ts"], st["wv_sb"]
                z_sb = st["z_sb"]
                # partition reduction + broadcast: one bf16 ones-matmul per stat
                ps_S = psst.tile([P, CT], f32, name="ps_S", tag="psS")
                ps_Q = psst.tile([P, CT], f32, name="ps_Q", tag="psQ")
                nc.tensor.matmul(ps_S[:, :ct], ones_bf[:], st["sacc"][:, :ct])
                nc.tensor.matmul(ps_Q[:, :ct], ones_bf[:], st["qacc"][:, :ct])
                mean32 = stp.tile([P, CT], f32, name="mean32", tag="mean32")[:, :ct]
                nc.vector.tensor_scalar_mul(mean32, ps_S[:, :ct], 1.0 / D)
                var32 = stp.tile([P, CT], f32, name="var32", tag="var32")[:, :ct]
                nc.vector.tensor_scalar_mul(var32, ps_Q[:, :ct], 1.0 / D)
                msq = stp.tile([P, CT], f32, name="msq", tag="msq")[:, :ct]
                nc.vector.tensor_tensor(msq, mean32, mean32, OP.mult)
                nc.vector.tensor_tensor(var32, var32, msq, OP.subtract)
                # rw = wv * rsqrt(var + eps). Abs_reciprocal_sqrt is a single
                # ACT function in a single table set (var+eps > 0, so the abs
                # is a no-op); Ln+Exp spans two sets and pays two extra
                # 1.28us table loads per tile.
                rs = stp.tile([P, CT], f32, name="rs", tag="rs")[:, :ct]
                nc.scalar.activation(
                    out=rs, in_=var32, func=AF.Abs_reciprocal_sqrt,
                    bias=eps_sb[:], scale=1.0,
                )
                nc.vector.tensor_tensor(rs, rs, wv_sb, OP.mult)
                rwb = stp.tile([P, CT], bf16, name="rwb", tag="rwb")[:, :ct]
                nc.vector.tensor_copy(rwb, rs)
                meanb = stp.tile([P, CT], bf16, name="meanb", tag="meanb")[:, :ct]
                nc.vector.tensor_copy(meanb, mean32)
                # normalize + scale; two whole-tile broadcast DVE ops (the
                # per-op ~150-cycle init amortizes 8x vs per-chunk ops),
                # stage the whole tile, store with one DMA
                oc = ocp.tile([P, MO2, CT], bf16, name="oc")
                if not (use_lng or use_lnb):
                    a0, a1 = bass.broadcast_tensor_aps(
                        z_sb[:, :, :ct], meanb[:, None, :]
                    )
                    nc.vector.tensor_tensor(oc[:, :, :ct], a0, a1, OP.subtract)
                    a0, a1 = bass.broadcast_tensor_aps(
                        oc[:, :, :ct], rwb[:, None, :]
                    )
                    nc.vector.tensor_tensor(oc[:, :, :ct], a0, a1, OP.mult)
                else:
                    for mo in range(MO2):
                        d = oc[:, mo, :ct]
                        nc.vector.tensor_tensor(d, z_sb[:, mo, :ct], meanb, OP.subtract)
                        nc.vector.tensor_tensor(d, d, rwb, OP.mult)
                        if use_lng:
                            nc.vector.tensor_scalar_mul(d, d, lng_sb[:, mo : mo + 1])
                        if use_lnb:
                            lb = stp.tile([P, CT], f32, name="lb", tag="lb")[:, :ct]
                            nc.vector.tensor_scalar_mul(lb, wv_sb, lnb_sb[:, mo : mo + 1])
                            nc.vector.tensor_tensor(d, d, lb, OP.add)
                nc.sync.dma_start(out_r[:, :, ts], oc[:, :, :ct])

            prev_st = None
            for t, (t0, ct) in enumerate(tiles):
                hT_sb = emit_mm1(t, ct, prev_st)
                if prev_st is not None:
                    # emitted after the full mm1 so the ACT table switch for
                    # the rsqrt never sits between gelus the next mm2 needs
                    ln_tail(prev_st)
                # prefetch token tiles two ahead (x8 for mm1, xb for residual)
                for tn_i in ([1, 2] if t == 0 else [t + 2]):
                    if tn_i < len(tiles):
                        tn, cn = tiles[tn_i]
                        nc.sync.dma_start(
                            x8_tiles[tn_i % 3][:, :, :cn], x8_r[:, :, tn : tn + cn]
                        )
                        nc.sync.dma_start(
                            xb_tiles[tn_i % 3][:, :, :cn], xb_r[:, :, tn : tn + cn]
                        )
                prev_st = mm2_begin(t, t0, ct, hT_sb)
            for mo in range(MO2):
                mm2_group(prev_st, mo)
            ln_tail(prev_st)

    nc.finalize()
    return nc


def _route(x, gate_w):
    """Host gate: top-2 per token + softmax combine weights (matches
    jax.lax.top_k tie-breaking: lower index wins)."""
    xt = x.reshape(-1, D)
    scores = xt.astype(np.float32) @ gate_w.astype(np.float32)  # [T, E]
    e0 = np.argmax(scores, axis=1)
    s0 = scores[np.arange(T), e0]
    masked = scores.copy()
    masked[np.arange(T), e0] = -np.inf
    e1 = np.argmax(masked, axis=1)
    s1 = masked[np.arange(T), e1]
    mx = np.maximum(s0, s1)
    z0 = np.exp((s0 - mx).astype(np.float64))
    z1 = np.exp((s1 - mx).astype(np.float64))
    den = z0 + z1
    w0 = (z0 / den).astype(np.float32)
    w1 = (z1 / den).astype(np.float32)
    return xt, e0, e1, w0, w1


def kernel(x, gate_w, w1, b1, w2, b2, ln_g, ln_b):
    from concourse.bass_utils import run_bass_kernel_spmd

    x = np.asarray(x)
    xt, e0, e1, wk0, wk1 = _route(x, np.asarray(gate_w))

    # slot assignment: expert e's token list = tokens with e0==e, then e1==e
    idx_e, wv_e = [], []
    for e in range(E):
        i0 = np.nonzero(e0 == e)[0]
        i1 = np.nonzero(e1 == e)[0]
        idx_e.append(np.concatenate([i0, i1]))
        wv_e.append(np.concatenate([wk0[i0], wk1[i1]]))
    maxn = max(len(i) for i in idx_e)
    C = max(PAD, -(-maxn // PAD) * PAD)

    use_b2 = bool(np.any(np.asarray(b2) != 0))
    use_lng = bool(np.any(np.asarray(ln_g) != 1))
    use_lnb = bool(np.any(np.asarray(ln_b) != 0))
    key = (C, use_b2, use_lng, use_lnb)
    if key not in _kernel_cache:
        _kernel_cache[key] = _build_bass(C, use_b2, use_lng, use_lnb)
    nc = _kernel_cache[key]

    f8 = ml_dtypes.float8_e4m3
    bf = ml_dtypes.bfloat16

    def chunked(a, n):  # [n*P] -> [P, n] host prelayout
        return np.ascontiguousarray(np.asarray(a, np.float32).reshape(n, P).T)

    in_maps = []
    for e in range(E):
        n = len(idx_e[e])
        xTe = np.zeros((D, C), np.float32)
        xTe[:, :n] = xt[idx_e[e]].T
        wve = np.zeros((C,), np.float32)
        wve[:n] = wv_e[e]
        im = {
            "x8": xTe.astype(f8),
            "xb": xTe.astype(bf),
            "w1": np.ascontiguousarray(np.asarray(w1)[e]).astype(f8),
            "w2": np.ascontiguousarray(np.asarray(w2)[e]).astype(f8),
            "b1": chunked(np.asarray(b1)[e], MO1),
            "wv": np.broadcast_to(wve, (P, C)).copy(),
        }
        if use_b2:
            im["b2"] = chunked(np.asarray(b2)[e], MO2)
        if use_lng:
            im["ln_g"] = chunked(np.asarray(ln_g)[e], MO2)
        if use_lnb:
            im["ln_b"] = chunked(np.asarray(ln_b)[e], MO2)
        in_maps.append(im)

    res = run_bass_kernel_spmd(nc, in_maps, core_ids=list(range(E)))
    kernel.last_results = res

    # combine: token t's two contributions live at known (expert, slot) pairs
    slot0 = np.empty(T, np.int64)
    slot1 = np.empty(T, np.int64)
    for e in range(E):
        n0 = int(np.sum(e0 == e))
        slot0[e0 == e] = np.arange(n0)
        slot1[e1 == e] = n0 + np.arange(int(np.sum(e1 == e)))
    Y = np.stack([res.results[e]["outT"].astype(np.float32) for e in range(E)])
    out = Y[e0, :, slot0] + Y[e1, :, slot1]  # [T, D]
    return out.reshape(x.shape).astype(np.float32)
